# revision 1
# baseline (speedup 1.0000x reference)
"""Causal attention (q/k/v proj + post-softmax-mask renorm attention) on 8
Trainium2 NeuronCores, two SPMD Bass launches, bf16 data plane.

Launch 1 (uniform): d_out-sharded QKV projections in bf16. Core c computes
qT/kT/vT slices [256, 4096] bf16 for its d_out slice from streamed bf16 xT
and host-transposed bf16 weight slices. psum accumulation fp32, outputs
rounded to bf16 (attention tolerance is 2e-2; bf16 rounding costs ~0.3%).

Launch 2 (8 variants via tc.Switch on partition id): causal attention over
q-row blocks, zigzag balanced (core c owns rows [256c,+256) u [256(15-c),
+256)). Scores computed TRANSPOSED (kT-tile stationary, packed qT moving):
psum sT[j, i] directly matches the AV stationary layout, so no PE
transposes and no DVE copies. Diagonal tiles get an additive -1e9 maskT in
psum; exp on ACT writes bf16 eT tiles; row sums accumulate on the PE via
single-shot ones-vector matmuls into per-(u,jt) psum columns (interleaved
multi-matmul accumulation groups within one psum bank lose writes on HW),
reduced on DVE; AV accumulates over j with 8 psum banks in two d-halves;
1/rowsum applied at psum->sbuf; bf16 out, host upcasts.

Schedule note: the ~30us DMA ramp before the first scores matmul acts as
k-chunk prefetch; attempts to start the PE earlier just moved the stall
into mid-phase (HBM ~340 GB/s/core is the binding resource, and phase-1
k demand is ~294 GB/s). The v stream self-throttles via its tile pool.

Softmax note: reference computes full softmax then masks + renormalizes;
the full-softmax denominator cancels, so this equals causal softmax
computed directly (exp without max subtraction is safe: |scores/sqrt(d)|
<= ~3.5 for these input scales).
"""
import os
import numpy as np
import ml_dtypes

import concourse.bacc as bacc
import concourse.mybir as mybir
import concourse.tile as tile
from concourse.bass_utils import run_bass_kernel_spmd

BF16 = mybir.dt.bfloat16
F32 = mybir.dt.float32
NP_BF16 = ml_dtypes.bfloat16

S, D = 4096, 2048
DS = D // 8            # 256: per-core d_out slice (launch 1)
NT = D // 128          # 16 contraction tiles
SCALE = 1.0 / np.sqrt(D)

_cache = {}
last_exec_ns = {}      # filled when BASS_KERNEL_TRACE=1 (test.py)


def _trace_on():
    return os.environ.get("BASS_KERNEL_TRACE", "") == "1"


def _ceil_div(a, b):
    return -(-a // b)


def _build_l1():
    nc = bacc.Bacc("TRN2", target_bir_lowering=False, debug=False)
    d_xT = nc.dram_tensor("xT", [D, S], BF16, kind="ExternalInput")
    d_wq = nc.dram_tensor("wqT", [D, DS], BF16, kind="ExternalInput")
    d_wk = nc.dram_tensor("wkT", [D, DS], BF16, kind="ExternalInput")
    d_wv = nc.dram_tensor("wvT", [D, DS], BF16, kind="ExternalInput")
    d_qT = nc.dram_tensor("qT", [DS, S], BF16, kind="ExternalOutput")
    d_kT = nc.dram_tensor("kT", [DS, S], BF16, kind="ExternalOutput")
    d_v = nc.dram_tensor("vT", [DS, S], BF16, kind="ExternalOutput")

    NB = S // 1024
    with tile.TileContext(nc) as tc:
        with (
            tc.tile_pool(name="w", bufs=1) as wp,
            tc.tile_pool(name="xb", bufs=2) as xp,
            tc.tile_pool(name="ob", bufs=4) as op,
            tc.tile_pool(name="pqk", bufs=4, space="PSUM") as pqk,
        ):
            NC = 4
            w_tiles = {}
            for wi, d_w in ((0, d_wq), (1, d_wk), (2, d_wv)):
                for tc4 in range(NC):
                    wt = wp.tile([128, 4, DS], BF16, tag=f"w{wi}_{tc4}")
                    w_tiles[wi, tc4] = wt

            def load_xb(b):
                tiles = []
                for tc4 in range(NC):
                    t_x = xp.tile([128, 4, 1024], BF16, tag=f"xb{tc4}")
                    xeng = nc.scalar if tc4 % 2 else nc.sync
                    xeng.dma_start(
                        t_x[:],
                        d_xT[tc4 * 512:(tc4 + 1) * 512,
                             b * 1024:(b + 1) * 1024].rearrange(
                            "(t p) i -> p t i", p=128))
                    tiles.append(t_x)
                return tiles

            for tc4 in range(NC):
                weng = nc.sync if tc4 % 2 == 0 else nc.scalar
                weng.dma_start(
                    w_tiles[0, tc4][:],
                    d_wq[tc4 * 512:(tc4 + 1) * 512, :].rearrange(
                        "(t p) d -> p t d", p=128))
            xb0 = load_xb(0)
            for tc4 in range(NC):
                e1 = nc.sync if tc4 % 2 == 0 else nc.scalar
                e2 = nc.scalar if tc4 % 2 == 0 else nc.sync
                e1.dma_start(
                    w_tiles[1, tc4][:],
                    d_wk[tc4 * 512:(tc4 + 1) * 512, :].rearrange(
                        "(t p) d -> p t d", p=128))
                e2.dma_start(
                    w_tiles[2, tc4][:],
                    d_wv[tc4 * 512:(tc4 + 1) * 512, :].rearrange(
                        "(t p) d -> p t d", p=128))

            cp = 0
            for b in range(NB):
                x_tiles = xb0 if b == 0 else load_xb(b)
                for wi, d_o in ((0, d_qT), (1, d_kT), (2, d_v)):
                    for dp in range(DS // 128):
                        p_a = pqk.tile([128, 512], F32, tag="p_a")
                        p_b = pqk.tile([128, 512], F32, tag="p_b")
                        for t in range(NT):
                            w_tile = w_tiles[wi, t // 4][:, t % 4,
                                                         dp * 128:(dp + 1) * 128]
                            nc.tensor.matmul(
                                p_a[:], w_tile, x_tiles[t // 4][:, t % 4, 0:512],
                                start=(t == 0), stop=(t == NT - 1))
                            nc.tensor.matmul(
                                p_b[:], w_tile, x_tiles[t // 4][:, t % 4, 512:1024],
                                start=(t == 0), stop=(t == NT - 1))
                        for h, p_h in ((0, p_a), (1, p_b)):
                            t_o = op.tile([128, 512], BF16, tag="oqk")
                            if cp % 2 == 0:
                                nc.vector.tensor_copy(t_o[:], p_h[:])
                            else:
                                nc.scalar.copy(t_o[:], p_h[:])
                            cp += 1
                            nc.gpsimd.dma_start(
                                d_o[dp * 128:(dp + 1) * 128,
                                    b * 1024 + h * 512: b * 1024 + (h + 1) * 512],
                                t_o[:])
    nc.compile()
    return nc


def _build_l2():
    nc = bacc.Bacc("TRN2", target_bir_lowering=False, debug=False)
    # qT packed [p, t(16), i(512)]; kT packed [p, ch(8), t(16), j(512)]
    d_q = nc.dram_tensor("qp", [128, NT * 512], BF16, kind="ExternalInput")
    d_k = nc.dram_tensor("kp", [128, (S // 512) * NT * 512], BF16,
                         kind="ExternalInput")
    d_v = nc.dram_tensor("v", [S, D], BF16, kind="ExternalInput")
    d_maskT = nc.dram_tensor("maskT", [128, 128], F32, kind="ExternalInput")
    d_ones = nc.dram_tensor("ones", [128, 1], BF16, kind="ExternalInput")
    d_out = nc.dram_tensor("out", [512, D], BF16, kind="ExternalOutput")

    with tile.TileContext(nc) as tc:
        with (
            tc.tile_pool(name="cst", bufs=1) as cst,
            tc.tile_pool(name="qp", bufs=1) as qp,
            tc.tile_pool(name="kc", bufs=3) as kcp,
            tc.tile_pool(name="vc", bufs=16) as vcp,
            tc.tile_pool(name="et", bufs=1) as etp,
            tc.tile_pool(name="sm", bufs=1) as smp,
            tc.tile_pool(name="ob", bufs=4) as obp,
        ):
            t_maskT = cst.tile([128, 128], F32, tag="maskT")
            t_ones = cst.tile([128, 1], BF16, tag="ones")
            nc.gpsimd.dma_start(t_maskT[:], d_maskT.ap())
            nc.gpsimd.dma_start(t_ones[:], d_ones.ap())
            t_q = qp.tile([128, NT * 512], BF16, tag="qT")
            for g in range(4):
                qeng = nc.sync if g % 2 == 0 else nc.scalar
                qeng.dma_start(t_q[:, g * 2048:(g + 1) * 2048],
                               d_q[:, g * 2048:(g + 1) * 2048])

            pid = nc.partition_id()
            for c in tc.Switch(pid, 8):
                lim = [2 * c + 1, 2 * c + 2, 31 - 2 * c, 32 - 2 * c]
                LIMX = lim[3]
                NCH = _ceil_div(LIMX, 4)
                NPJ = _ceil_div(LIMX, 2)

                t_eT = etp.tile([128, 32, 512], BF16, tag="eT")

                # ---- v prefetch: all (half, pair) chunks, gpsimd queue;
                # the 16-slot pool caps how far it runs ahead ----
                v_tiles = []
                for half in range(2):
                    for jp in range(NPJ):
                        npair = min(2, LIMX - jp * 2)
                        tv = vcp.tile([128, 2, 1024], BF16, tag="vc")
                        nc.gpsimd.dma_start(
                            tv[:, :npair, :],
                            d_v[jp * 256: jp * 256 + npair * 128,
                                half * 1024:(half + 1) * 1024].rearrange(
                                "(n p) d -> p n d", p=128))
                        v_tiles.append(tv)

                # ---- phase 1: scores^T -> exp(bf16 eT) + PE row-sums ----
                ph1 = tc.tile_pool(name=f"ps{c}", bufs=4, space="PSUM")
                psp = ph1.__enter__()
                ph1r = tc.tile_pool(name=f"pr{c}", bufs=1, space="PSUM")
                prp = ph1r.__enter__()
                # per-(u, jt) row-sum partials; single-shot matmuls only —
                # interleaved multi-matmul accumulation groups within one
                # psum bank lose writes on HW.
                p_rs = prp.tile([128, 4 * 32], F32, tag="p_rs")
                for ch in range(NCH):
                    t_kc = kcp.tile([128, NT * 512], BF16, tag="kc")
                    keng = nc.sync if ch % 2 == 0 else nc.scalar
                    keng.dma_start(t_kc[:],
                                   d_k[:, ch * 8192:(ch + 1) * 8192])
                    for jq in range(4):
                        jt = ch * 4 + jq
                        if jt >= LIMX:
                            break
                        u_min = next(u for u in range(4) if lim[u] > jt)
                        ioff = 128 * u_min
                        p_s = psp.tile([128, 512], F32, tag="p_s")
                        for t in range(NT):
                            nc.tensor.matmul(
                                p_s[:, ioff:512],
                                t_kc[:, t * 512 + jq * 128:
                                     t * 512 + jq * 128 + 128],
                                t_q[:, t * 512 + ioff: t * 512 + 512],
                                start=(t == 0), stop=(t == NT - 1))
                        for u in range(u_min, 4):
                            if lim[u] - 1 == jt:
                                nc.vector.tensor_add(
                                    p_s[:, u * 128:(u + 1) * 128],
                                    p_s[:, u * 128:(u + 1) * 128],
                                    t_maskT[:])
                        nc.scalar.activation(
                            t_eT[:, jt, ioff:512], p_s[:, ioff:512],
                            mybir.ActivationFunctionType.Exp,
                            scale=SCALE)
                        for u in range(u_min, 4):
                            nc.tensor.matmul(
                                p_rs[:, u * 32 + jt: u * 32 + jt + 1],
                                t_eT[:, jt, u * 128:(u + 1) * 128],
                                t_ones[:],
                                start=True, stop=True)

                t_sum = smp.tile([128, 4], F32, tag="sum")
                for u in range(4):
                    nc.vector.reduce_sum(
                        t_sum[:, u:u + 1],
                        p_rs[:, u * 32: u * 32 + lim[u]],
                        axis=mybir.AxisListType.X)
                t_rec = smp.tile([128, 4], F32, tag="rec")
                nc.vector.reciprocal(t_rec[:], t_sum[:])
                ph1r.__exit__(None, None, None)
                ph1.__exit__(None, None, None)

                # ---- phase 2: AV over j, two d-halves, 8-bank psum ----
                ph2 = tc.tile_pool(name=f"po{c}", bufs=1, space="PSUM")
                pop = ph2.__enter__()
                for half in range(2):
                    p_out = {}
                    for u in range(4):
                        for db in range(2):
                            p_o = pop.tile([128, 512], F32, tag=f"po{u}{db}")
                            p_out[u, db] = p_o
                    for jt in range(LIMX):
                        tv = v_tiles[half * NPJ + jt // 2]
                        sl = jt % 2
                        for u in range(4):
                            if jt >= lim[u]:
                                continue
                            for db in range(2):
                                nc.tensor.matmul(
                                    p_out[u, db][:],
                                    t_eT[:, jt, u * 128:(u + 1) * 128],
                                    tv[:, sl, db * 512:(db + 1) * 512],
                                    start=(jt == 0),
                                    stop=(jt == lim[u] - 1))
                    for u in range(4):
                        for db in range(2):
                            t_o = obp.tile([128, 512], BF16, tag="t_o")
                            nc.vector.tensor_scalar_mul(
                                t_o[:], p_out[u, db][:], t_rec[:, u:u + 1])
                            oeng = nc.sync if (u + db) % 2 == 0 else nc.scalar
                            oeng.dma_start(
                                d_out[u * 128:(u + 1) * 128,
                                      half * 1024 + db * 512:
                                      half * 1024 + (db + 1) * 512],
                                t_o[:])
                ph2.__exit__(None, None, None)
    nc.compile()
    return nc


def kernel(x, W_q, W_k, W_v):
    x = np.asarray(x, dtype=np.float32)
    W_q = np.asarray(W_q, dtype=np.float32)
    W_k = np.asarray(W_k, dtype=np.float32)
    W_v = np.asarray(W_v, dtype=np.float32)
    if "l1" not in _cache:
        _cache["l1"] = _build_l1()
    if "l2" not in _cache:
        _cache["l2"] = _build_l2()
    nc1, nc2 = _cache["l1"], _cache["l2"]
    trace = _trace_on()

    # ---- launch 1: QKV projections (bf16) ----
    xT = np.ascontiguousarray(x.T).astype(NP_BF16)
    WqT = np.ascontiguousarray(W_q.T).astype(NP_BF16)
    WkT = np.ascontiguousarray(W_k.T).astype(NP_BF16)
    WvT = np.ascontiguousarray(W_v.T).astype(NP_BF16)
    in_maps = []
    for c in range(8):
        sl = slice(c * DS, (c + 1) * DS)
        in_maps.append({
            "xT": xT,
            "wqT": np.ascontiguousarray(WqT[:, sl]),
            "wkT": np.ascontiguousarray(WkT[:, sl]),
            "wvT": np.ascontiguousarray(WvT[:, sl]),
        })
    res1 = run_bass_kernel_spmd(nc1, in_maps, core_ids=list(range(8)),
                                trace=trace)
    qT = np.vstack([res1.results[c]["qT"] for c in range(8)])
    kT = np.vstack([res1.results[c]["kT"] for c in range(8)])
    v = np.ascontiguousarray(
        np.vstack([res1.results[c]["vT"] for c in range(8)]).T)

    # ---- launch 2: causal attention ----
    # kT packed [p, ch, t, jw]: kp[p, ch*8192 + t*512 + jw] = kT[t*128+p,
    # ch*512+jw]
    kp = np.ascontiguousarray(
        kT.reshape(NT, 128, S // 512, 512).transpose(1, 2, 0, 3)
        .reshape(128, (S // 512) * NT * 512))
    jj = np.arange(128)[:, None]
    ii = np.arange(128)[None, :]
    maskT = np.where(jj > ii, -1e9, 0.0).astype(np.float32)
    ones = np.ones((128, 1), dtype=NP_BF16)
    in_maps2 = []
    for c in range(8):
        lo, hi = 256 * c, 256 * (15 - c)
        q_own = np.concatenate([qT[:, lo:lo + 256], qT[:, hi:hi + 256]],
                               axis=1)
        qp = np.ascontiguousarray(
            q_own.reshape(NT, 128, 512).transpose(1, 0, 2).reshape(128, -1))
        in_maps2.append({
            "qp": qp, "kp": kp, "v": v, "maskT": maskT, "ones": ones,
        })
    res2 = run_bass_kernel_spmd(nc2, in_maps2, core_ids=list(range(8)),
                                trace=trace)
    out = np.empty((S, D), np.float32)
    for c in range(8):
        lo, hi = 256 * c, 256 * (15 - c)
        blk = res2.results[c]["out"].astype(np.float32)
        out[lo:lo + 256] = blk[0:256]
        out[hi:hi + 256] = blk[256:512]

    if trace:
        last_exec_ns["l1"] = res1.exec_time_ns
        last_exec_ns["l2"] = res2.exec_time_ns
        last_exec_ns["res1"] = res1
        last_exec_ns["res2"] = res2
    return out



# revision 2
# speedup vs baseline: 1.2068x; 1.2068x over previous
"""Causal attention (q/k/v proj + post-softmax-mask renorm attention) on 8
Trainium2 NeuronCores, two SPMD Bass launches, bf16 compute plane with
fp8-e4m3 q/k storage between launches (bandwidth only — all matmul math
runs at bf16-equivalent precision through fp32 psum).

Launch 1 (uniform): d_out-sharded QKV projections. Core c computes qT/kT
slices [256, 4096] in fp8 (W_q/W_k host-prescaled by 32 so q*32 spans
~±94, well inside TRN e4m3's ±240) and vT slice [256, 4096] bf16, from
streamed bf16 xT and host-transposed bf16 weight slices. First wq tile
and first x tile are issued on separate queues ahead of everything else
so the PE starts ~3us in instead of ~18us.

Launch 2 (8 variants via tc.Switch on partition id): causal attention
over q-row blocks, zigzag balanced (core c owns rows [256c,+256) u
[256(15-c),+256)). Scores computed TRANSPOSED (fp8 kT-tile stationary,
packed fp8 qT moving; psum fp32): psum sT[j, i] matches the AV
stationary layout, so no PE transposes. Exp activation scale folds the
1/sqrt(d) and the 2^-10 from the two 32x weight scalings. Diagonal
tiles get an additive -1e9 maskT in psum; exp on ACT writes bf16 eT
tiles; row sums accumulate on the PE via single-shot ones-vector
matmuls into per-(u,jt) psum columns (interleaved multi-matmul
accumulation groups within one psum bank lose writes on HW), reduced on
DVE; AV (bf16 e, bf16 v) accumulates over j with 8 psum banks in two
d-halves; 1/rowsum applied at psum->sbuf; bf16 out, host upcasts.

Softmax note: reference computes full softmax then masks + renorms; the
full-softmax denominator cancels, so this equals causal softmax
computed directly (exp without max subtraction is safe: |scores/
sqrt(d)| <= ~3.5 for these input scales). fp8 storage of q/k adds
~1.3% rms score-exponent noise -> ~9e-3 on the max-err metric
(host-simulated), vs the 2e-2 gate.
"""
import os
import numpy as np
import ml_dtypes

import concourse.bacc as bacc
import concourse.mybir as mybir
import concourse.tile as tile
from concourse.bass_utils import run_bass_kernel_spmd

BF16 = mybir.dt.bfloat16
F8 = mybir.dt.float8e4
F32 = mybir.dt.float32
NP_BF16 = ml_dtypes.bfloat16
NP_F8 = ml_dtypes.float8_e4m3

S, D = 4096, 2048
DS = D // 8            # 256: per-core d_out slice (launch 1)
NT = D // 128          # 16 contraction tiles
WS = 32.0              # host prescale on W_q/W_k (power of 2, exact in bf16)
SCALE = 1.0 / (np.sqrt(D) * WS * WS)

_cache = {}
last_exec_ns = {}      # filled when BASS_KERNEL_TRACE=1 (test.py)


def _trace_on():
    return os.environ.get("BASS_KERNEL_TRACE", "") == "1"


def _ceil_div(a, b):
    return -(-a // b)


def _build_l1():
    nc = bacc.Bacc("TRN2", target_bir_lowering=False, debug=False)
    d_xT = nc.dram_tensor("xT", [D, S], BF16, kind="ExternalInput")
    d_wq = nc.dram_tensor("wqT", [D, DS], BF16, kind="ExternalInput")
    d_wk = nc.dram_tensor("wkT", [D, DS], BF16, kind="ExternalInput")
    d_wv = nc.dram_tensor("wvT", [D, DS], BF16, kind="ExternalInput")
    d_qT = nc.dram_tensor("qT", [DS, S], F8, kind="ExternalOutput")
    d_kT = nc.dram_tensor("kT", [DS, S], F8, kind="ExternalOutput")
    d_v = nc.dram_tensor("vT", [DS, S], BF16, kind="ExternalOutput")

    NB = S // 1024
    with tile.TileContext(nc) as tc:
        with (
            tc.tile_pool(name="w", bufs=1) as wp,
            tc.tile_pool(name="xb", bufs=2) as xp,
            tc.tile_pool(name="ob", bufs=4) as op,
            tc.tile_pool(name="pqk", bufs=4, space="PSUM") as pqk,
        ):
            NC = 4
            w_tiles = {}
            for wi, d_w in ((0, d_wq), (1, d_wk), (2, d_wv)):
                for tc4 in range(NC):
                    wt = wp.tile([128, 4, DS], BF16, tag=f"w{wi}_{tc4}")
                    w_tiles[wi, tc4] = wt

            def load_xb(b, eng0=None):
                tiles = []
                for tc4 in range(NC):
                    t_x = xp.tile([128, 4, 1024], BF16, tag=f"xb{tc4}")
                    if eng0 is not None:
                        xeng = eng0
                    else:
                        xeng = nc.scalar if tc4 % 2 else nc.sync
                    xeng.dma_start(
                        t_x[:],
                        d_xT[tc4 * 512:(tc4 + 1) * 512,
                             b * 1024:(b + 1) * 1024].rearrange(
                            "(t p) i -> p t i", p=128))
                    tiles.append(t_x)
                return tiles

            # ramp: first matmul needs only wq[0] + xb0[0]; put them at the
            # head of two separate queues so the PE starts ~3us in.
            nc.sync.dma_start(
                w_tiles[0, 0][:],
                d_wq[0:512, :].rearrange("(t p) d -> p t d", p=128))
            xb0 = load_xb(0, eng0=nc.scalar)
            for tc4 in range(1, NC):
                nc.sync.dma_start(
                    w_tiles[0, tc4][:],
                    d_wq[tc4 * 512:(tc4 + 1) * 512, :].rearrange(
                        "(t p) d -> p t d", p=128))
            for tc4 in range(NC):
                e1 = nc.sync if tc4 % 2 == 0 else nc.scalar
                e2 = nc.scalar if tc4 % 2 == 0 else nc.sync
                e1.dma_start(
                    w_tiles[1, tc4][:],
                    d_wk[tc4 * 512:(tc4 + 1) * 512, :].rearrange(
                        "(t p) d -> p t d", p=128))
                e2.dma_start(
                    w_tiles[2, tc4][:],
                    d_wv[tc4 * 512:(tc4 + 1) * 512, :].rearrange(
                        "(t p) d -> p t d", p=128))

            cp = 0
            for b in range(NB):
                x_tiles = xb0 if b == 0 else load_xb(b)
                for wi, d_o, o_dt in ((0, d_qT, F8), (1, d_kT, F8),
                                      (2, d_v, BF16)):
                    for dp in range(DS // 128):
                        p_a = pqk.tile([128, 512], F32, tag="p_a")
                        p_b = pqk.tile([128, 512], F32, tag="p_b")
                        for t in range(NT):
                            w_tile = w_tiles[wi, t // 4][:, t % 4,
                                                         dp * 128:(dp + 1) * 128]
                            nc.tensor.matmul(
                                p_a[:], w_tile, x_tiles[t // 4][:, t % 4, 0:512],
                                start=(t == 0), stop=(t == NT - 1))
                            nc.tensor.matmul(
                                p_b[:], w_tile, x_tiles[t // 4][:, t % 4, 512:1024],
                                start=(t == 0), stop=(t == NT - 1))
                        for h, p_h in ((0, p_a), (1, p_b)):
                            t_o = op.tile([128, 512], o_dt, tag=f"oqk{o_dt}")
                            if cp % 2 == 0:
                                nc.vector.tensor_copy(t_o[:], p_h[:])
                            else:
                                nc.scalar.copy(t_o[:], p_h[:])
                            cp += 1
                            nc.gpsimd.dma_start(
                                d_o[dp * 128:(dp + 1) * 128,
                                    b * 1024 + h * 512: b * 1024 + (h + 1) * 512],
                                t_o[:])
    nc.compile()
    return nc


def _build_l2():
    nc = bacc.Bacc("TRN2", target_bir_lowering=False, debug=False)
    # qT packed [p, t(16), i(512)] fp8; kT packed [p, ch(8), t(16), j(512)] fp8
    d_q = nc.dram_tensor("qp", [128, NT * 512], F8, kind="ExternalInput")
    d_k = nc.dram_tensor("kp", [128, (S // 512) * NT * 512], F8,
                         kind="ExternalInput")
    d_v = nc.dram_tensor("v", [S, D], BF16, kind="ExternalInput")
    d_maskT = nc.dram_tensor("maskT", [128, 128], F32, kind="ExternalInput")
    d_ones = nc.dram_tensor("ones", [128, 1], BF16, kind="ExternalInput")
    d_out = nc.dram_tensor("out", [512, D], BF16, kind="ExternalOutput")

    with tile.TileContext(nc) as tc:
        with (
            tc.tile_pool(name="cst", bufs=1) as cst,
            tc.tile_pool(name="qp", bufs=1) as qp,
            tc.tile_pool(name="kc", bufs=3) as kcp,
            tc.tile_pool(name="vc", bufs=16) as vcp,
            tc.tile_pool(name="et", bufs=1) as etp,
            tc.tile_pool(name="sm", bufs=1) as smp,
            tc.tile_pool(name="ob", bufs=4) as obp,
        ):
            t_maskT = cst.tile([128, 128], F32, tag="maskT")
            t_ones = cst.tile([128, 1], BF16, tag="ones")
            nc.gpsimd.dma_start(t_maskT[:], d_maskT.ap())
            nc.gpsimd.dma_start(t_ones[:], d_ones.ap())
            t_q = qp.tile([128, NT * 512], F8, tag="qT")
            for g in range(4):
                qeng = nc.sync if g % 2 == 0 else nc.scalar
                qeng.dma_start(t_q[:, g * 2048:(g + 1) * 2048],
                               d_q[:, g * 2048:(g + 1) * 2048])

            pid = nc.partition_id()
            for c in tc.Switch(pid, 8):
                lim = [2 * c + 1, 2 * c + 2, 31 - 2 * c, 32 - 2 * c]
                LIMX = lim[3]
                NCH = _ceil_div(LIMX, 4)
                NPJ = _ceil_div(LIMX, 2)

                t_eT = etp.tile([128, 32, 512], BF16, tag="eT")

                # ---- v prefetch: all (half, pair) chunks, gpsimd queue;
                # the 16-slot pool caps how far it runs ahead ----
                v_tiles = []
                for half in range(2):
                    for jp in range(NPJ):
                        npair = min(2, LIMX - jp * 2)
                        tv = vcp.tile([128, 2, 1024], BF16, tag="vc")
                        nc.gpsimd.dma_start(
                            tv[:, :npair, :],
                            d_v[jp * 256: jp * 256 + npair * 128,
                                half * 1024:(half + 1) * 1024].rearrange(
                                "(n p) d -> p n d", p=128))
                        v_tiles.append(tv)

                # ---- phase 1: scores^T -> exp(bf16 eT) + PE row-sums ----
                ph1 = tc.tile_pool(name=f"ps{c}", bufs=4, space="PSUM")
                psp = ph1.__enter__()
                ph1r = tc.tile_pool(name=f"pr{c}", bufs=1, space="PSUM")
                prp = ph1r.__enter__()
                # per-(u, jt) row-sum partials; single-shot matmuls only —
                # interleaved multi-matmul accumulation groups within one
                # psum bank lose writes on HW.
                p_rs = prp.tile([128, 4 * 32], F32, tag="p_rs")
                for ch in range(NCH):
                    t_kc = kcp.tile([128, NT * 512], F8, tag="kc")
                    keng = nc.sync if ch % 2 == 0 else nc.scalar
                    keng.dma_start(t_kc[:],
                                   d_k[:, ch * 8192:(ch + 1) * 8192])
                    for jq in range(4):
                        jt = ch * 4 + jq
                        if jt >= LIMX:
                            break
                        u_min = next(u for u in range(4) if lim[u] > jt)
                        ioff = 128 * u_min
                        p_s = psp.tile([128, 512], F32, tag="p_s")
                        for t in range(NT):
                            nc.tensor.matmul(
                                p_s[:, ioff:512],
                                t_kc[:, t * 512 + jq * 128:
                                     t * 512 + jq * 128 + 128],
                                t_q[:, t * 512 + ioff: t * 512 + 512],
                                start=(t == 0), stop=(t == NT - 1))
                        for u in range(u_min, 4):
                            if lim[u] - 1 == jt:
                                nc.vector.tensor_add(
                                    p_s[:, u * 128:(u + 1) * 128],
                                    p_s[:, u * 128:(u + 1) * 128],
                                    t_maskT[:])
                        nc.scalar.activation(
                            t_eT[:, jt, ioff:512], p_s[:, ioff:512],
                            mybir.ActivationFunctionType.Exp,
                            scale=SCALE)
                        for u in range(u_min, 4):
                            nc.tensor.matmul(
                                p_rs[:, u * 32 + jt: u * 32 + jt + 1],
                                t_eT[:, jt, u * 128:(u + 1) * 128],
                                t_ones[:],
                                start=True, stop=True)

                t_sum = smp.tile([128, 4], F32, tag="sum")
                for u in range(4):
                    nc.vector.reduce_sum(
                        t_sum[:, u:u + 1],
                        p_rs[:, u * 32: u * 32 + lim[u]],
                        axis=mybir.AxisListType.X)
                t_rec = smp.tile([128, 4], F32, tag="rec")
                nc.vector.reciprocal(t_rec[:], t_sum[:])
                ph1r.__exit__(None, None, None)
                ph1.__exit__(None, None, None)

                # ---- phase 2: AV over j, two d-halves, 8-bank psum ----
                ph2 = tc.tile_pool(name=f"po{c}", bufs=1, space="PSUM")
                pop = ph2.__enter__()
                for half in range(2):
                    p_out = {}
                    for u in range(4):
                        for db in range(2):
                            p_o = pop.tile([128, 512], F32, tag=f"po{u}{db}")
                            p_out[u, db] = p_o
                    for jt in range(LIMX):
                        tv = v_tiles[half * NPJ + jt // 2]
                        sl = jt % 2
                        for u in range(4):
                            if jt >= lim[u]:
                                continue
                            for db in range(2):
                                nc.tensor.matmul(
                                    p_out[u, db][:],
                                    t_eT[:, jt, u * 128:(u + 1) * 128],
                                    tv[:, sl, db * 512:(db + 1) * 512],
                                    start=(jt == 0),
                                    stop=(jt == lim[u] - 1))
                    for u in range(4):
                        for db in range(2):
                            t_o = obp.tile([128, 512], BF16, tag="t_o")
                            nc.vector.tensor_scalar_mul(
                                t_o[:], p_out[u, db][:], t_rec[:, u:u + 1])
                            oeng = nc.sync if (u + db) % 2 == 0 else nc.scalar
                            oeng.dma_start(
                                d_out[u * 128:(u + 1) * 128,
                                      half * 1024 + db * 512:
                                      half * 1024 + (db + 1) * 512],
                                t_o[:])
                ph2.__exit__(None, None, None)
    nc.compile()
    return nc


def kernel(x, W_q, W_k, W_v):
    x = np.asarray(x, dtype=np.float32)
    W_q = np.asarray(W_q, dtype=np.float32)
    W_k = np.asarray(W_k, dtype=np.float32)
    W_v = np.asarray(W_v, dtype=np.float32)
    if "l1" not in _cache:
        _cache["l1"] = _build_l1()
    if "l2" not in _cache:
        _cache["l2"] = _build_l2()
    nc1, nc2 = _cache["l1"], _cache["l2"]
    trace = _trace_on()

    # ---- launch 1: QKV projections (bf16 in, fp8 q/k out) ----
    xT = np.ascontiguousarray(x.T).astype(NP_BF16)
    WqT = np.ascontiguousarray(W_q.T * WS).astype(NP_BF16)
    WkT = np.ascontiguousarray(W_k.T * WS).astype(NP_BF16)
    WvT = np.ascontiguousarray(W_v.T).astype(NP_BF16)
    in_maps = []
    for c in range(8):
        sl = slice(c * DS, (c + 1) * DS)
        in_maps.append({
            "xT": xT,
            "wqT": np.ascontiguousarray(WqT[:, sl]),
            "wkT": np.ascontiguousarray(WkT[:, sl]),
            "wvT": np.ascontiguousarray(WvT[:, sl]),
        })
    res1 = run_bass_kernel_spmd(nc1, in_maps, core_ids=list(range(8)),
                                trace=trace)
    qT = np.vstack([res1.results[c]["qT"] for c in range(8)])
    kT = np.vstack([res1.results[c]["kT"] for c in range(8)])
    v = np.ascontiguousarray(
        np.vstack([res1.results[c]["vT"] for c in range(8)]).T)

    # ---- launch 2: causal attention ----
    # kT packed [p, ch, t, jw]: kp[p, ch*8192 + t*512 + jw] = kT[t*128+p,
    # ch*512+jw]
    kp = np.ascontiguousarray(
        kT.reshape(NT, 128, S // 512, 512).transpose(1, 2, 0, 3)
        .reshape(128, (S // 512) * NT * 512))
    jj = np.arange(128)[:, None]
    ii = np.arange(128)[None, :]
    maskT = np.where(jj > ii, -1e9, 0.0).astype(np.float32)
    ones = np.ones((128, 1), dtype=NP_BF16)
    in_maps2 = []
    for c in range(8):
        lo, hi = 256 * c, 256 * (15 - c)
        q_own = np.concatenate([qT[:, lo:lo + 256], qT[:, hi:hi + 256]],
                               axis=1)
        qp = np.ascontiguousarray(
            q_own.reshape(NT, 128, 512).transpose(1, 0, 2).reshape(128, -1))
        in_maps2.append({
            "qp": qp, "kp": kp, "v": v, "maskT": maskT, "ones": ones,
        })
    res2 = run_bass_kernel_spmd(nc2, in_maps2, core_ids=list(range(8)),
                                trace=trace)
    out = np.empty((S, D), np.float32)
    for c in range(8):
        lo, hi = 256 * c, 256 * (15 - c)
        blk = res2.results[c]["out"].astype(np.float32)
        out[lo:lo + 256] = blk[0:256]
        out[hi:hi + 256] = blk[256:512]

    if trace:
        last_exec_ns["l1"] = res1.exec_time_ns
        last_exec_ns["l2"] = res2.exec_time_ns
        last_exec_ns["res1"] = res1
        last_exec_ns["res2"] = res2
    return out


# revision 8
# speedup vs baseline: 1.2090x; 1.0018x over previous
"""Causal attention (q/k/v proj + post-softmax-mask renorm attention) on 8
Trainium2 NeuronCores, two SPMD Bass launches, bf16 compute plane with
fp8-e4m3 q/k storage between launches (bandwidth only — all matmul math
runs at bf16-equivalent precision through fp32 psum).

Launch 1 (uniform): d_out-sharded QKV projections. Core c computes qT/kT
slices [256, 4096] in fp8 (W_q/W_k host-prescaled by 32 so q*32 spans
~±94, well inside TRN e4m3's ±240) and vT slice [256, 4096] bf16, from
streamed bf16 xT and host-transposed bf16 weight slices. First wq tile
and first x tile are issued on separate queues ahead of everything else
so the PE starts ~3us in instead of ~18us.

Launch 2 (8 variants via tc.Switch on partition id): causal attention
over q-row blocks, zigzag balanced (core c owns rows [256c,+256) u
[256(15-c),+256)). Scores computed TRANSPOSED (fp8 kT-tile stationary,
packed fp8 qT moving; psum fp32): psum sT[j, i] matches the AV
stationary layout, so no PE transposes. Exp activation scale folds the
1/sqrt(d) and the 2^-10 from the two 32x weight scalings. Diagonal
tiles get an additive -1e9 maskT in psum; exp on ACT writes bf16 eT
tiles; row sums accumulate on the PE via single-shot ones-vector
matmuls into per-(u,jt) psum columns (interleaved multi-matmul
accumulation groups within one psum bank lose writes on HW), reduced on
DVE; AV (bf16 e, bf16 v) accumulates over j with 8 psum banks in two
d-halves; 1/rowsum applied at psum->sbuf; bf16 out, host upcasts.

Softmax note: reference computes full softmax then masks + renorms; the
full-softmax denominator cancels, so this equals causal softmax
computed directly (exp without max subtraction is safe: |scores/
sqrt(d)| <= ~3.5 for these input scales). fp8 storage of q/k adds
~1.3% rms score-exponent noise -> ~9e-3 on the max-err metric
(host-simulated), vs the 2e-2 gate.
"""
import os
import numpy as np
import ml_dtypes

import concourse.bacc as bacc
import concourse.mybir as mybir
import concourse.tile as tile
from concourse.bass_utils import run_bass_kernel_spmd

BF16 = mybir.dt.bfloat16
F8 = mybir.dt.float8e4
F32 = mybir.dt.float32
NP_BF16 = ml_dtypes.bfloat16
NP_F8 = ml_dtypes.float8_e4m3

S, D = 4096, 2048
DS = D // 8            # 256: per-core d_out slice (launch 1)
NT = D // 128          # 16 contraction tiles
WS = 32.0              # host prescale on W_q/W_k (power of 2, exact in bf16)
SCALE = 1.0 / (np.sqrt(D) * WS * WS)

_cache = {}
last_exec_ns = {}      # filled when BASS_KERNEL_TRACE=1 (test.py)


def _trace_on():
    return os.environ.get("BASS_KERNEL_TRACE", "") == "1"


def _ceil_div(a, b):
    return -(-a // b)


def _build_l1():
    nc = bacc.Bacc("TRN2", target_bir_lowering=False, debug=False)
    d_xT = nc.dram_tensor("xT", [D, S], BF16, kind="ExternalInput")
    d_wq = nc.dram_tensor("wqT", [D, DS], BF16, kind="ExternalInput")
    d_wk = nc.dram_tensor("wkT", [D, DS], BF16, kind="ExternalInput")
    d_wv = nc.dram_tensor("wvT", [D, DS], BF16, kind="ExternalInput")
    d_qT = nc.dram_tensor("qT", [DS, S], F8, kind="ExternalOutput")
    d_kT = nc.dram_tensor("kT", [DS, S], F8, kind="ExternalOutput")
    d_v = nc.dram_tensor("vT", [DS, S], BF16, kind="ExternalOutput")

    NB = S // 1024
    with tile.TileContext(nc) as tc:
        with (
            tc.tile_pool(name="w", bufs=1) as wp,
            tc.tile_pool(name="xb", bufs=2) as xp,
            tc.tile_pool(name="ob", bufs=4) as op,
            tc.tile_pool(name="pqk", bufs=4, space="PSUM") as pqk,
        ):
            NC = 4
            w_tiles = {}
            for wi, d_w in ((0, d_wq), (1, d_wk), (2, d_wv)):
                for tc4 in range(NC):
                    wt = wp.tile([128, 4, DS], BF16, tag=f"w{wi}_{tc4}")
                    w_tiles[wi, tc4] = wt

            def load_xb(b):
                tiles = []
                for tc4 in range(NC):
                    t_x = xp.tile([128, 4, 1024], BF16, tag=f"xb{tc4}")
                    xeng = nc.scalar if tc4 % 2 else nc.sync
                    xeng.dma_start(
                        t_x[:],
                        d_xT[tc4 * 512:(tc4 + 1) * 512,
                             b * 1024:(b + 1) * 1024].rearrange(
                            "(t p) i -> p t i", p=128))
                    tiles.append(t_x)
                return tiles

            # ramp: interleave wq tiles and xb0 tiles across the two HWDGE
            # rings in consumption order (t ascending needs wq[t//4] and
            # xb0[t//4]); wk/wv follow — needed one/two psum-groups later.
            def load_w(wi, tc4, eng):
                d_w = (d_wq, d_wk, d_wv)[wi]
                eng.dma_start(
                    w_tiles[wi, tc4][:],
                    d_w[tc4 * 512:(tc4 + 1) * 512, :].rearrange(
                        "(t p) d -> p t d", p=128))

            xb0 = []

            def load_x0(tc4, eng):
                t_x = xp.tile([128, 4, 1024], BF16, tag=f"xb{tc4}")
                eng.dma_start(
                    t_x[:],
                    d_xT[tc4 * 512:(tc4 + 1) * 512, 0:1024].rearrange(
                        "(t p) i -> p t i", p=128))
                xb0.append(t_x)

            load_w(0, 0, nc.sync)
            load_x0(0, nc.scalar)
            load_w(0, 1, nc.scalar)
            load_x0(1, nc.sync)
            load_w(0, 2, nc.sync)
            load_x0(2, nc.scalar)
            load_w(0, 3, nc.scalar)
            load_x0(3, nc.sync)
            for tc4 in range(NC):
                load_w(1, tc4, nc.sync if tc4 % 2 == 0 else nc.scalar)
            for tc4 in range(NC):
                load_w(2, tc4, nc.scalar if tc4 % 2 == 0 else nc.sync)

            cp = 0
            for b in range(NB):
                x_tiles = xb0 if b == 0 else load_xb(b)
                for wi, d_o, o_dt in ((0, d_qT, F8), (1, d_kT, F8),
                                      (2, d_v, BF16)):
                    for dp in range(DS // 128):
                        p_a = pqk.tile([128, 512], F32, tag="p_a")
                        p_b = pqk.tile([128, 512], F32, tag="p_b")
                        for t in range(NT):
                            w_tile = w_tiles[wi, t // 4][:, t % 4,
                                                         dp * 128:(dp + 1) * 128]
                            nc.tensor.matmul(
                                p_a[:], w_tile, x_tiles[t // 4][:, t % 4, 0:512],
                                start=(t == 0), stop=(t == NT - 1))
                            nc.tensor.matmul(
                                p_b[:], w_tile, x_tiles[t // 4][:, t % 4, 512:1024],
                                start=(t == 0), stop=(t == NT - 1))
                        t_o = op.tile([128, 1024], o_dt, tag=f"oqk{o_dt}")
                        for h, p_h in ((0, p_a), (1, p_b)):
                            if cp % 2 == 0:
                                nc.vector.tensor_copy(
                                    t_o[:, h * 512:(h + 1) * 512], p_h[:])
                            else:
                                nc.scalar.copy(
                                    t_o[:, h * 512:(h + 1) * 512], p_h[:])
                            cp += 1
                        nc.gpsimd.dma_start(
                            d_o[dp * 128:(dp + 1) * 128,
                                b * 1024:(b + 1) * 1024],
                            t_o[:])
    nc.compile()
    return nc


def _build_l2():
    nc = bacc.Bacc("TRN2", target_bir_lowering=False, debug=False)
    # qT packed [p, t(16), i(512)] fp8; kT packed [p, ch(8), t(16), j(512)] fp8
    d_q = nc.dram_tensor("qp", [128, NT * 512], F8, kind="ExternalInput")
    d_k = nc.dram_tensor("kp", [128, (S // 512) * NT * 512], F8,
                         kind="ExternalInput")
    d_v = nc.dram_tensor("v", [S, D], BF16, kind="ExternalInput")
    d_maskT = nc.dram_tensor("maskT", [128, 128], F32, kind="ExternalInput")
    d_ones = nc.dram_tensor("ones", [128, 1], BF16, kind="ExternalInput")
    d_out = nc.dram_tensor("out", [512, D], BF16, kind="ExternalOutput")

    with tile.TileContext(nc) as tc:
        with (
            tc.tile_pool(name="cst", bufs=1) as cst,
            tc.tile_pool(name="qp", bufs=1) as qp,
            tc.tile_pool(name="kc", bufs=3) as kcp,
            tc.tile_pool(name="vc", bufs=16) as vcp,
            tc.tile_pool(name="et", bufs=1) as etp,
            tc.tile_pool(name="sm", bufs=1) as smp,
            tc.tile_pool(name="ob", bufs=4) as obp,
        ):
            t_maskT = cst.tile([128, 128], F32, tag="maskT")
            t_ones = cst.tile([128, 1], BF16, tag="ones")
            nc.gpsimd.dma_start(t_maskT[:], d_maskT.ap())
            nc.gpsimd.dma_start(t_ones[:], d_ones.ap())
            t_q = qp.tile([128, NT * 512], F8, tag="qT")
            for g in range(4):
                qeng = nc.sync if g % 2 == 0 else nc.scalar
                qeng.dma_start(t_q[:, g * 2048:(g + 1) * 2048],
                               d_q[:, g * 2048:(g + 1) * 2048])

            pid = nc.partition_id()
            for c in tc.Switch(pid, 8):
                lim = [2 * c + 1, 2 * c + 2, 31 - 2 * c, 32 - 2 * c]
                LIMX = lim[3]
                NCH = _ceil_div(LIMX, 4)
                NPJ = _ceil_div(LIMX, 2)

                t_eT = etp.tile([128, 32, 512], BF16, tag="eT")

                # kc chunk 0 rides the gpsimd ring AHEAD of the v-prefetch
                # flood (FIFO per ring), so the first scores matmul isn't
                # starved by 8MB of queued v traffic.
                t_kc0 = kcp.tile([128, NT * 512], F8, tag="kc")
                nc.gpsimd.dma_start(t_kc0[:], d_k[:, 0:8192])

                # ---- v prefetch: all (half, pair) chunks, gpsimd queue;
                # the 16-slot pool caps how far it runs ahead ----
                v_tiles = []
                for half in range(2):
                    for jp in range(NPJ):
                        npair = min(2, LIMX - jp * 2)
                        tv = vcp.tile([128, 2, 1024], BF16, tag="vc")
                        nc.gpsimd.dma_start(
                            tv[:, :npair, :],
                            d_v[jp * 256: jp * 256 + npair * 128,
                                half * 1024:(half + 1) * 1024].rearrange(
                                "(n p) d -> p n d", p=128))
                        v_tiles.append(tv)

                # ---- phase 1: scores^T -> exp(bf16 eT) + PE row-sums ----
                ph1 = tc.tile_pool(name=f"ps{c}", bufs=4, space="PSUM")
                psp = ph1.__enter__()
                ph1r = tc.tile_pool(name=f"pr{c}", bufs=1, space="PSUM")
                prp = ph1r.__enter__()
                # per-(u, jt) row-sum partials; single-shot matmuls only —
                # interleaved multi-matmul accumulation groups within one
                # psum bank lose writes on HW.
                p_rs = prp.tile([128, 4 * 32], F32, tag="p_rs")
                for ch in range(NCH):
                    if ch == 0:
                        t_kc = t_kc0
                    else:
                        t_kc = kcp.tile([128, NT * 512], F8, tag="kc")
                        keng = nc.sync if ch % 2 == 1 else nc.scalar
                        keng.dma_start(t_kc[:],
                                       d_k[:, ch * 8192:(ch + 1) * 8192])
                    for jq in range(4):
                        jt = ch * 4 + jq
                        if jt >= LIMX:
                            break
                        u_min = next(u for u in range(4) if lim[u] > jt)
                        ioff = 128 * u_min
                        p_s = psp.tile([128, 512], F32, tag="p_s")
                        for t in range(NT):
                            nc.tensor.matmul(
                                p_s[:, ioff:512],
                                t_kc[:, t * 512 + jq * 128:
                                     t * 512 + jq * 128 + 128],
                                t_q[:, t * 512 + ioff: t * 512 + 512],
                                start=(t == 0), stop=(t == NT - 1))
                        for u in range(u_min, 4):
                            if lim[u] - 1 == jt:
                                nc.vector.tensor_add(
                                    p_s[:, u * 128:(u + 1) * 128],
                                    p_s[:, u * 128:(u + 1) * 128],
                                    t_maskT[:])
                        nc.scalar.activation(
                            t_eT[:, jt, ioff:512], p_s[:, ioff:512],
                            mybir.ActivationFunctionType.Exp,
                            scale=SCALE)
                        for u in range(u_min, 4):
                            nc.tensor.matmul(
                                p_rs[:, u * 32 + jt: u * 32 + jt + 1],
                                t_eT[:, jt, u * 128:(u + 1) * 128],
                                t_ones[:],
                                start=True, stop=True)

                t_sum = smp.tile([128, 4], F32, tag="sum")
                for u in range(4):
                    nc.vector.reduce_sum(
                        t_sum[:, u:u + 1],
                        p_rs[:, u * 32: u * 32 + lim[u]],
                        axis=mybir.AxisListType.X)
                t_rec = smp.tile([128, 4], F32, tag="rec")
                nc.vector.reciprocal(t_rec[:], t_sum[:])
                ph1r.__exit__(None, None, None)
                ph1.__exit__(None, None, None)

                # ---- phase 2: AV over j, two d-halves, 8-bank psum ----
                ph2 = tc.tile_pool(name=f"po{c}", bufs=1, space="PSUM")
                pop = ph2.__enter__()
                for half in range(2):
                    p_out = {}
                    for u in range(4):
                        for db in range(2):
                            p_o = pop.tile([128, 512], F32, tag=f"po{u}{db}")
                            p_out[u, db] = p_o
                    for jt in range(LIMX):
                        tv = v_tiles[half * NPJ + jt // 2]
                        sl = jt % 2
                        for u in range(4):
                            if jt >= lim[u]:
                                continue
                            for db in range(2):
                                nc.tensor.matmul(
                                    p_out[u, db][:],
                                    t_eT[:, jt, u * 128:(u + 1) * 128],
                                    tv[:, sl, db * 512:(db + 1) * 512],
                                    start=(jt == 0),
                                    stop=(jt == lim[u] - 1))
                    for u in range(4):
                        t_o = obp.tile([128, 1024], BF16, tag="t_o")
                        for db in range(2):
                            if (u + db) % 2 == 0:
                                nc.vector.tensor_scalar_mul(
                                    t_o[:, db * 512:(db + 1) * 512],
                                    p_out[u, db][:], t_rec[:, u:u + 1])
                            else:
                                nc.scalar.activation(
                                    t_o[:, db * 512:(db + 1) * 512],
                                    p_out[u, db][:],
                                    mybir.ActivationFunctionType.Identity,
                                    scale=t_rec[:, u:u + 1])
                        oeng = nc.sync if u % 2 == 0 else nc.scalar
                        oeng.dma_start(
                            d_out[u * 128:(u + 1) * 128,
                                  half * 1024:(half + 1) * 1024],
                            t_o[:])
                ph2.__exit__(None, None, None)
    nc.compile()
    return nc


def kernel(x, W_q, W_k, W_v):
    x = np.asarray(x, dtype=np.float32)
    W_q = np.asarray(W_q, dtype=np.float32)
    W_k = np.asarray(W_k, dtype=np.float32)
    W_v = np.asarray(W_v, dtype=np.float32)
    if "l1" not in _cache:
        _cache["l1"] = _build_l1()
    if "l2" not in _cache:
        _cache["l2"] = _build_l2()
    nc1, nc2 = _cache["l1"], _cache["l2"]
    trace = _trace_on()

    # ---- launch 1: QKV projections (bf16 in, fp8 q/k out) ----
    xT = np.ascontiguousarray(x.T).astype(NP_BF16)
    WqT = np.ascontiguousarray(W_q.T * WS).astype(NP_BF16)
    WkT = np.ascontiguousarray(W_k.T * WS).astype(NP_BF16)
    WvT = np.ascontiguousarray(W_v.T).astype(NP_BF16)
    in_maps = []
    for c in range(8):
        sl = slice(c * DS, (c + 1) * DS)
        in_maps.append({
            "xT": xT,
            "wqT": np.ascontiguousarray(WqT[:, sl]),
            "wkT": np.ascontiguousarray(WkT[:, sl]),
            "wvT": np.ascontiguousarray(WvT[:, sl]),
        })
    res1 = run_bass_kernel_spmd(nc1, in_maps, core_ids=list(range(8)),
                                trace=trace)
    qT = np.vstack([res1.results[c]["qT"] for c in range(8)])
    kT = np.vstack([res1.results[c]["kT"] for c in range(8)])
    v = np.ascontiguousarray(
        np.vstack([res1.results[c]["vT"] for c in range(8)]).T)

    # ---- launch 2: causal attention ----
    # kT packed [p, ch, t, jw]: kp[p, ch*8192 + t*512 + jw] = kT[t*128+p,
    # ch*512+jw]
    kp = np.ascontiguousarray(
        kT.reshape(NT, 128, S // 512, 512).transpose(1, 2, 0, 3)
        .reshape(128, (S // 512) * NT * 512))
    jj = np.arange(128)[:, None]
    ii = np.arange(128)[None, :]
    maskT = np.where(jj > ii, -1e9, 0.0).astype(np.float32)
    ones = np.ones((128, 1), dtype=NP_BF16)
    in_maps2 = []
    for c in range(8):
        lo, hi = 256 * c, 256 * (15 - c)
        q_own = np.concatenate([qT[:, lo:lo + 256], qT[:, hi:hi + 256]],
                               axis=1)
        qp = np.ascontiguousarray(
            q_own.reshape(NT, 128, 512).transpose(1, 0, 2).reshape(128, -1))
        in_maps2.append({
            "qp": qp, "kp": kp, "v": v, "maskT": maskT, "ones": ones,
        })
    res2 = run_bass_kernel_spmd(nc2, in_maps2, core_ids=list(range(8)),
                                trace=trace)
    out = np.empty((S, D), np.float32)
    for c in range(8):
        lo, hi = 256 * c, 256 * (15 - c)
        blk = res2.results[c]["out"].astype(np.float32)
        out[lo:lo + 256] = blk[0:256]
        out[hi:hi + 256] = blk[256:512]

    if trace:
        last_exec_ns["l1"] = res1.exec_time_ns
        last_exec_ns["l2"] = res2.exec_time_ns
        last_exec_ns["res1"] = res1
        last_exec_ns["res2"] = res2
    return out


# revision 15
# speedup vs baseline: 1.2092x; 1.0002x over previous
"""Causal attention (q/k/v proj + post-softmax-mask renorm attention) on 8
Trainium2 NeuronCores, two SPMD Bass launches, bf16 compute plane with
fp8-e4m3 q/k storage between launches (bandwidth only — all matmul math
runs at bf16-equivalent precision through fp32 psum).

Launch 1 (uniform): d_out-sharded QKV projections. Core c computes qT/kT
slices [256, 4096] in fp8 (W_q/W_k host-prescaled by 32 so q*32 spans
~±94, well inside TRN e4m3's ±240) and vT slice [256, 4096] bf16, from
streamed bf16 xT and host-transposed bf16 weight slices. First wq tile
and first x tile are issued on separate queues ahead of everything else
so the PE starts ~3us in instead of ~18us.

Launch 2 (8 variants via tc.Switch on partition id): causal attention
over q-row blocks, zigzag balanced (core c owns rows [256c,+256) u
[256(15-c),+256)). Scores computed TRANSPOSED (fp8 kT-tile stationary,
packed fp8 qT moving; psum fp32): psum sT[j, i] matches the AV
stationary layout, so no PE transposes. Exp activation scale folds the
1/sqrt(d) and the 2^-10 from the two 32x weight scalings. Diagonal
tiles get an additive -1e9 maskT in psum; exp on ACT writes bf16 eT
tiles; row sums accumulate on the PE via single-shot ones-vector
matmuls into per-(u,jt) psum columns (interleaved multi-matmul
accumulation groups within one psum bank lose writes on HW), reduced on
DVE; AV (bf16 e, bf16 v) accumulates over j with 8 psum banks in two
d-halves; 1/rowsum applied at psum->sbuf; bf16 out, host upcasts.

Softmax note: reference computes full softmax then masks + renorms; the
full-softmax denominator cancels, so this equals causal softmax
computed directly (exp without max subtraction is safe: |scores/
sqrt(d)| <= ~3.5 for these input scales). fp8 storage of q/k adds
~1.3% rms score-exponent noise -> ~9e-3 on the max-err metric
(host-simulated), vs the 2e-2 gate.
"""
import os
import numpy as np
import ml_dtypes

import concourse.bacc as bacc
import concourse.mybir as mybir
import concourse.tile as tile
from concourse.bass_utils import run_bass_kernel_spmd

BF16 = mybir.dt.bfloat16
F8 = mybir.dt.float8e4
F32 = mybir.dt.float32
NP_BF16 = ml_dtypes.bfloat16
NP_F8 = ml_dtypes.float8_e4m3

S, D = 4096, 2048
DS = D // 8            # 256: per-core d_out slice (launch 1)
NT = D // 128          # 16 contraction tiles
WS = 32.0              # host prescale on W_q/W_k (power of 2, exact in bf16)
SCALE = 1.0 / (np.sqrt(D) * WS * WS)

_cache = {}
last_exec_ns = {}      # filled when BASS_KERNEL_TRACE=1 (test.py)


def _trace_on():
    return os.environ.get("BASS_KERNEL_TRACE", "") == "1"


def _ceil_div(a, b):
    return -(-a // b)


def _build_l1():
    nc = bacc.Bacc("TRN2", target_bir_lowering=False, debug=False)
    d_xT = nc.dram_tensor("xT", [D, S], BF16, kind="ExternalInput")
    d_wq = nc.dram_tensor("wqT", [D, DS], BF16, kind="ExternalInput")
    d_wk = nc.dram_tensor("wkT", [D, DS], BF16, kind="ExternalInput")
    d_wv = nc.dram_tensor("wvT", [D, DS], BF16, kind="ExternalInput")
    d_qT = nc.dram_tensor("qT", [DS, S], F8, kind="ExternalOutput")
    d_kT = nc.dram_tensor("kT", [DS, S], F8, kind="ExternalOutput")
    d_v = nc.dram_tensor("vT", [DS, S], BF16, kind="ExternalOutput")

    NB = S // 1024
    with tile.TileContext(nc) as tc:
        with (
            tc.tile_pool(name="w", bufs=1) as wp,
            tc.tile_pool(name="xb", bufs=2) as xp,
            tc.tile_pool(name="ob", bufs=4) as op,
            tc.tile_pool(name="pqk", bufs=4, space="PSUM") as pqk,
        ):
            NC = 4
            w_tiles = {}
            for wi, d_w in ((0, d_wq), (1, d_wk), (2, d_wv)):
                for tc4 in range(NC):
                    wt = wp.tile([128, 4, DS], BF16, tag=f"w{wi}_{tc4}")
                    w_tiles[wi, tc4] = wt

            def load_xb(b):
                tiles = []
                for tc4 in range(NC):
                    t_x = xp.tile([128, 4, 1024], BF16, tag=f"xb{tc4}")
                    xeng = nc.scalar if tc4 % 2 else nc.sync
                    xeng.dma_start(
                        t_x[:],
                        d_xT[tc4 * 512:(tc4 + 1) * 512,
                             b * 1024:(b + 1) * 1024].rearrange(
                            "(t p) i -> p t i", p=128))
                    tiles.append(t_x)
                return tiles

            # ramp: interleave wq tiles and xb0 tiles across the two HWDGE
            # rings in consumption order (t ascending needs wq[t//4] and
            # xb0[t//4]); wk/wv follow — needed one/two psum-groups later.
            def load_w(wi, tc4, eng):
                d_w = (d_wq, d_wk, d_wv)[wi]
                eng.dma_start(
                    w_tiles[wi, tc4][:],
                    d_w[tc4 * 512:(tc4 + 1) * 512, :].rearrange(
                        "(t p) d -> p t d", p=128))

            xb0 = []

            def load_x0(tc4, eng):
                t_x = xp.tile([128, 4, 1024], BF16, tag=f"xb{tc4}")
                eng.dma_start(
                    t_x[:],
                    d_xT[tc4 * 512:(tc4 + 1) * 512, 0:1024].rearrange(
                        "(t p) i -> p t i", p=128))
                xb0.append(t_x)

            load_w(0, 0, nc.sync)
            load_x0(0, nc.scalar)
            load_w(0, 1, nc.scalar)
            load_x0(1, nc.sync)
            load_w(0, 2, nc.sync)
            load_x0(2, nc.scalar)
            load_w(0, 3, nc.scalar)
            load_x0(3, nc.sync)
            for tc4 in range(NC):
                load_w(1, tc4, nc.sync if tc4 % 2 == 0 else nc.scalar)
            for tc4 in range(NC):
                load_w(2, tc4, nc.scalar if tc4 % 2 == 0 else nc.sync)

            cp = 0
            for b in range(NB):
                x_tiles = xb0 if b == 0 else load_xb(b)
                for wi, d_o, o_dt in ((0, d_qT, F8), (1, d_kT, F8),
                                      (2, d_v, BF16)):
                    t_o = op.tile([128, 2, 1024], o_dt, tag=f"oqk{o_dt}")
                    for dp in range(DS // 128):
                        p_a = pqk.tile([128, 512], F32, tag="p_a")
                        p_b = pqk.tile([128, 512], F32, tag="p_b")
                        for t in range(NT):
                            w_tile = w_tiles[wi, t // 4][:, t % 4,
                                                         dp * 128:(dp + 1) * 128]
                            nc.tensor.matmul(
                                p_a[:], w_tile, x_tiles[t // 4][:, t % 4, 0:512],
                                start=(t == 0), stop=(t == NT - 1))
                            nc.tensor.matmul(
                                p_b[:], w_tile, x_tiles[t // 4][:, t % 4, 512:1024],
                                start=(t == 0), stop=(t == NT - 1))
                        for h, p_h in ((0, p_a), (1, p_b)):
                            if cp % 2 == 0:
                                nc.vector.tensor_copy(
                                    t_o[:, dp, h * 512:(h + 1) * 512], p_h[:])
                            else:
                                nc.scalar.copy(
                                    t_o[:, dp, h * 512:(h + 1) * 512], p_h[:])
                            cp += 1
                    nc.gpsimd.dma_start(
                        d_o[:, b * 1024:(b + 1) * 1024].rearrange(
                            "(t p) i -> p t i", p=128),
                        t_o[:])
    nc.compile()
    return nc


def _build_l2():
    nc = bacc.Bacc("TRN2", target_bir_lowering=False, debug=False)
    # qT packed [p, t(16), i(512)] fp8; kT packed [p, ch(8), t(16), j(512)] fp8
    d_q = nc.dram_tensor("qp", [128, NT * 512], F8, kind="ExternalInput")
    d_k = nc.dram_tensor("kp", [128, (S // 512) * NT * 512], F8,
                         kind="ExternalInput")
    d_v = nc.dram_tensor("v", [S, D], BF16, kind="ExternalInput")
    d_maskT = nc.dram_tensor("maskT", [128, 128], F32, kind="ExternalInput")
    d_ones = nc.dram_tensor("ones", [128, 1], BF16, kind="ExternalInput")
    d_out = nc.dram_tensor("out", [512, D], BF16, kind="ExternalOutput")

    with tile.TileContext(nc) as tc:
        with (
            tc.tile_pool(name="cst", bufs=1) as cst,
            tc.tile_pool(name="qp", bufs=1) as qp,
            tc.tile_pool(name="kc", bufs=3) as kcp,
            tc.tile_pool(name="vc", bufs=10) as vcp,
            tc.tile_pool(name="et", bufs=1) as etp,
            tc.tile_pool(name="sm", bufs=1) as smp,
            tc.tile_pool(name="ob", bufs=4) as obp,
        ):
            t_maskT = cst.tile([128, 128], F32, tag="maskT")
            t_ones = cst.tile([128, 1], BF16, tag="ones")
            nc.gpsimd.dma_start(t_maskT[:], d_maskT.ap())
            nc.gpsimd.dma_start(t_ones[:], d_ones.ap())
            # q as two tiles (t<8 / t>=8) so the first 8 score matmuls only
            # wait on 512KB of q + 512KB of k.
            t_qh0 = qp.tile([128, 8 * 512], F8, tag="qT0")
            t_qh1 = qp.tile([128, 8 * 512], F8, tag="qT1")
            t_qh = [t_qh0, t_qh1]
            for h in (0, 1):
                nc.scalar.dma_start(t_qh[h][:],
                                    d_q[:, h * 4096:(h + 1) * 4096])

            def q_sl(t, lo, hi):
                return t_qh[t // 8][:, (t % 8) * 512 + lo:(t % 8) * 512 + hi]

            pid = nc.partition_id()
            for c in tc.Switch(pid, 8):
                lim = [2 * c + 1, 2 * c + 2, 31 - 2 * c, 32 - 2 * c]
                LIMX = lim[3]
                NCH = _ceil_div(LIMX, 4)
                NPJ = _ceil_div(LIMX, 2)

                t_eT = etp.tile([128, 32, 512], BF16, tag="eT")

                # kc chunk 0 leads the sync ring, split in two so the first
                # 8 score matmuls unlock after 512KB of k (+512KB of q).
                t_kc0 = kcp.tile([128, NT * 512], F8, tag="kc")
                nc.sync.dma_start(t_kc0[:, 0:4096], d_k[:, 0:4096])
                nc.sync.dma_start(t_kc0[:, 4096:8192], d_k[:, 4096:8192])

                # ---- v prefetch: 1MB tiles (4 j-subtiles), gpsimd queue;
                # SWDGE's ~0.8us/issue keeps it from flooding early HBM and
                # the pool caps how far it runs ahead ----
                NQJ = _ceil_div(LIMX, 4)
                v_tiles = []
                for half in range(2):
                    for jp in range(NQJ):
                        nquad = min(4, LIMX - jp * 4)
                        tv = vcp.tile([128, 4, 1024], BF16, tag="vc")
                        nc.gpsimd.dma_start(
                            tv[:, :nquad, :],
                            d_v[jp * 512: jp * 512 + nquad * 128,
                                half * 1024:(half + 1) * 1024].rearrange(
                                "(n p) d -> p n d", p=128))
                        v_tiles.append(tv)

                # ---- phase 1: scores^T -> exp(bf16 eT) + PE row-sums ----
                ph1 = tc.tile_pool(name=f"ps{c}", bufs=4, space="PSUM")
                psp = ph1.__enter__()
                ph1r = tc.tile_pool(name=f"pr{c}", bufs=1, space="PSUM")
                prp = ph1r.__enter__()
                # per-(u, jt) row-sum partials; single-shot matmuls only —
                # interleaved multi-matmul accumulation groups within one
                # psum bank lose writes on HW.
                p_rs = prp.tile([128, 4 * 32], F32, tag="p_rs")
                for ch in range(NCH):
                    if ch == 0:
                        t_kc = t_kc0
                    else:
                        t_kc = kcp.tile([128, NT * 512], F8, tag="kc")
                        keng = nc.sync if ch % 2 == 1 else nc.scalar
                        keng.dma_start(t_kc[:],
                                       d_k[:, ch * 8192:(ch + 1) * 8192])
                    for jq in range(4):
                        jt = ch * 4 + jq
                        if jt >= LIMX:
                            break
                        u_min = next(u for u in range(4) if lim[u] > jt)
                        ioff = 128 * u_min
                        p_s = psp.tile([128, 512], F32, tag="p_s")
                        for t in range(NT):
                            nc.tensor.matmul(
                                p_s[:, ioff:512],
                                t_kc[:, t * 512 + jq * 128:
                                     t * 512 + jq * 128 + 128],
                                q_sl(t, ioff, 512),
                                start=(t == 0), stop=(t == NT - 1))
                        for u in range(u_min, 4):
                            if lim[u] - 1 == jt:
                                nc.vector.tensor_add(
                                    p_s[:, u * 128:(u + 1) * 128],
                                    p_s[:, u * 128:(u + 1) * 128],
                                    t_maskT[:])
                        nc.scalar.activation(
                            t_eT[:, jt, ioff:512], p_s[:, ioff:512],
                            mybir.ActivationFunctionType.Exp,
                            scale=SCALE)
                        for u in range(u_min, 4):
                            nc.tensor.matmul(
                                p_rs[:, u * 32 + jt: u * 32 + jt + 1],
                                t_eT[:, jt, u * 128:(u + 1) * 128],
                                t_ones[:],
                                start=True, stop=True)

                t_sum = smp.tile([128, 4], F32, tag="sum")
                for u in range(4):
                    nc.vector.reduce_sum(
                        t_sum[:, u:u + 1],
                        p_rs[:, u * 32: u * 32 + lim[u]],
                        axis=mybir.AxisListType.X)
                t_rec = smp.tile([128, 4], F32, tag="rec")
                nc.vector.reciprocal(t_rec[:], t_sum[:])
                ph1r.__exit__(None, None, None)
                ph1.__exit__(None, None, None)

                # ---- phase 2: AV over j, two d-halves, 8-bank psum ----
                ph2 = tc.tile_pool(name=f"po{c}", bufs=1, space="PSUM")
                pop = ph2.__enter__()
                for half in range(2):
                    p_out = {}
                    for u in range(4):
                        for db in range(2):
                            p_o = pop.tile([128, 512], F32, tag=f"po{u}{db}")
                            p_out[u, db] = p_o
                    for jt in range(LIMX):
                        tv = v_tiles[half * NQJ + jt // 4]
                        sl = jt % 4
                        for u in range(4):
                            if jt >= lim[u]:
                                continue
                            for db in range(2):
                                nc.tensor.matmul(
                                    p_out[u, db][:],
                                    t_eT[:, jt, u * 128:(u + 1) * 128],
                                    tv[:, sl, db * 512:(db + 1) * 512],
                                    start=(jt == 0),
                                    stop=(jt == lim[u] - 1))
                    for u in range(4):
                        t_o = obp.tile([128, 1024], BF16, tag="t_o")
                        for db in range(2):
                            if (u + db) % 2 == 0:
                                nc.vector.tensor_scalar_mul(
                                    t_o[:, db * 512:(db + 1) * 512],
                                    p_out[u, db][:], t_rec[:, u:u + 1])
                            else:
                                nc.scalar.activation(
                                    t_o[:, db * 512:(db + 1) * 512],
                                    p_out[u, db][:],
                                    mybir.ActivationFunctionType.Identity,
                                    scale=t_rec[:, u:u + 1])
                        oeng = nc.sync if u % 2 == 0 else nc.scalar
                        oeng.dma_start(
                            d_out[u * 128:(u + 1) * 128,
                                  half * 1024:(half + 1) * 1024],
                            t_o[:])
                ph2.__exit__(None, None, None)
    nc.compile()
    return nc


def kernel(x, W_q, W_k, W_v):
    x = np.asarray(x, dtype=np.float32)
    W_q = np.asarray(W_q, dtype=np.float32)
    W_k = np.asarray(W_k, dtype=np.float32)
    W_v = np.asarray(W_v, dtype=np.float32)
    if "l1" not in _cache:
        _cache["l1"] = _build_l1()
    if "l2" not in _cache:
        _cache["l2"] = _build_l2()
    nc1, nc2 = _cache["l1"], _cache["l2"]
    trace = _trace_on()

    # ---- launch 1: QKV projections (bf16 in, fp8 q/k out) ----
    xT = np.ascontiguousarray(x.T).astype(NP_BF16)
    WqT = np.ascontiguousarray(W_q.T * WS).astype(NP_BF16)
    WkT = np.ascontiguousarray(W_k.T * WS).astype(NP_BF16)
    WvT = np.ascontiguousarray(W_v.T).astype(NP_BF16)
    in_maps = []
    for c in range(8):
        sl = slice(c * DS, (c + 1) * DS)
        in_maps.append({
            "xT": xT,
            "wqT": np.ascontiguousarray(WqT[:, sl]),
            "wkT": np.ascontiguousarray(WkT[:, sl]),
            "wvT": np.ascontiguousarray(WvT[:, sl]),
        })
    res1 = run_bass_kernel_spmd(nc1, in_maps, core_ids=list(range(8)),
                                trace=trace)
    qT = np.vstack([res1.results[c]["qT"] for c in range(8)])
    kT = np.vstack([res1.results[c]["kT"] for c in range(8)])
    v = np.ascontiguousarray(
        np.vstack([res1.results[c]["vT"] for c in range(8)]).T)

    # ---- launch 2: causal attention ----
    # kT packed [p, ch, t, jw]: kp[p, ch*8192 + t*512 + jw] = kT[t*128+p,
    # ch*512+jw]
    kp = np.ascontiguousarray(
        kT.reshape(NT, 128, S // 512, 512).transpose(1, 2, 0, 3)
        .reshape(128, (S // 512) * NT * 512))
    jj = np.arange(128)[:, None]
    ii = np.arange(128)[None, :]
    maskT = np.where(jj > ii, -1e9, 0.0).astype(np.float32)
    ones = np.ones((128, 1), dtype=NP_BF16)
    in_maps2 = []
    for c in range(8):
        lo, hi = 256 * c, 256 * (15 - c)
        q_own = np.concatenate([qT[:, lo:lo + 256], qT[:, hi:hi + 256]],
                               axis=1)
        qp = np.ascontiguousarray(
            q_own.reshape(NT, 128, 512).transpose(1, 0, 2).reshape(128, -1))
        in_maps2.append({
            "qp": qp, "kp": kp, "v": v, "maskT": maskT, "ones": ones,
        })
    res2 = run_bass_kernel_spmd(nc2, in_maps2, core_ids=list(range(8)),
                                trace=trace)
    out = np.empty((S, D), np.float32)
    for c in range(8):
        lo, hi = 256 * c, 256 * (15 - c)
        blk = res2.results[c]["out"].astype(np.float32)
        out[lo:lo + 256] = blk[0:256]
        out[hi:hi + 256] = blk[256:512]

    if trace:
        last_exec_ns["l1"] = res1.exec_time_ns
        last_exec_ns["l2"] = res2.exec_time_ns
        last_exec_ns["res1"] = res1
        last_exec_ns["res2"] = res2
    return out
